# revision 23
# baseline (speedup 1.0000x reference)
"""v5: unit-pipelined schedule over (batch, q-chunk) units.

Core math as v4 (bf16 datapath, fp8e4 DoubleRow scores with K single-rounded
and Q hi+lo compensated, 1/8 folded into wq). New in v5:
 - attention is a 4-unit pipeline over (b, qc): unit i runs scores+exp of
   qc_i on PE/ACT while the ctx matmuls of qc_{i-1} (reading its buffered
   exp tiles) and woven-in proj/transpose/outproj thunks fill PE slack.
   ACT (exp: 32x [128,1024] insts/unit) is the pacing engine.
 - PSUM plan (8 banks): sA,sB scores ping-pong [128,1024] (2+2);
   cA,cB ctx accumulation [65,512] halves (1+1); pA,pB proj/outproj/
   transpose scratch [128,512] (1+1).
 - ctx of a qc accumulates in two sequential 512-wide halves so it fits in
   2 banks; exp tiles are buffered across a whole unit (expp bufs=48).
 - projections emitted as 512-wide halves (1 bank each); weights/qt as in v4.
"""

import functools
from collections import deque
from contextlib import ExitStack

import numpy as np
import ml_dtypes

import concourse.bass as bass
import concourse.tile as tile
from concourse import mybir
from concourse.bass_utils import run_bass_kernel_spmd

B, S, D, H, DH = 2, 2048, 1024, 16, 64
N_CORES = 8
DPC = D // N_CORES
BS = B * S
NST = S // 128            # 16
NKT = D // 128            # 8

F32 = mybir.dt.float32
F32R = mybir.dt.float32r
BF16 = mybir.dt.bfloat16
F8 = mybir.dt.float8e4
DR = mybir.MatmulPerfMode.DoubleRow
Act = mybir.ActivationFunctionType
Alu = mybir.AluOpType


def _split_sync_commands(nc, max_waits=1, max_updates=8):
    for fn in nc.m.functions:
        for bb in fn.blocks:
            new_insts = []
            changed = False
            for inst in bb.instructions:
                si = getattr(inst, "sync_info", None)
                if si is not None:
                    waits = list(si.on_wait or [])
                    if len(waits) > max_waits:
                        for w in waits[:-max_waits]:
                            new_insts.append(mybir.InstNoOp(
                                name=nc.get_next_instruction_name(),
                                ins=[], outs=[], engine=inst.engine,
                                sync_info=mybir.SyncInfo(on_wait=[w], on_update=[]),
                            ))
                        si.on_wait = waits[-max_waits:]
                        changed = True
                    updates = list(si.on_update or [])
                    if len(updates) > max_updates:
                        si.on_update = updates[:max_updates]
                        new_insts.append(inst)
                        new_insts.append(mybir.InstNoOp(
                            name=nc.get_next_instruction_name(),
                            ins=[], outs=[], engine=inst.engine,
                            sync_info=mybir.SyncInfo(
                                on_wait=[], on_update=updates[max_updates:]),
                        ))
                        changed = True
                        continue
                new_insts.append(inst)
            if changed:
                bb.instructions = new_insts


def _dup2(ap):
    """lhsT slot-dup: [K, M] -> [K, 2, M] with stride-0 slot dim."""
    return bass.AP(tensor=ap.tensor, offset=ap.offset,
                   ap=[list(ap.ap[0]), [0, 2]] + [list(p) for p in ap.ap[1:]])


@functools.lru_cache(maxsize=2)
def _build(zero_bias=True):
    nc = bass.Bass()
    qt_d = nc.dram_tensor("qt", [D, BS], BF16, kind="ExternalInput")
    wq_d = nc.dram_tensor("wq", [128, NKT, DPC], BF16, kind="ExternalInput")
    wk_d = nc.dram_tensor("wk", [128, NKT, DPC], BF16, kind="ExternalInput")
    wv_d = nc.dram_tensor("wv", [128, NKT, DPC], BF16, kind="ExternalInput")
    bq_d = nc.dram_tensor("bq", [DPC, 1], F32, kind="ExternalInput")
    bk_d = nc.dram_tensor("bk", [DPC, 1], F32, kind="ExternalInput")
    bv_d = nc.dram_tensor("bv", [DPC, 1], F32, kind="ExternalInput")
    wo_d = nc.dram_tensor("wo", [DPC, D], BF16, kind="ExternalInput")
    out_d = nc.dram_tensor("out_part", [BS, D], BF16, kind="ExternalOutput")
    dn_d = nc.dram_tensor("dn_scratch", [2, S], BF16)
    ident_d = nc.inline_tensor(np.eye(128, dtype=ml_dtypes.bfloat16), "ident")

    with tile.TileContext(nc) as tc, ExitStack() as ctx:
        ctx.enter_context(nc.allow_low_precision(
            reason="fp8/bf16 intermediates validated against 2e-2 rel-err gate"))
        consts = ctx.enter_context(tc.tile_pool(name="consts", bufs=1))
        qt_pool = ctx.enter_context(tc.tile_pool(name="qt", bufs=1))
        proj = ctx.enter_context(tc.tile_pool(name="proj", bufs=2))
        vpool = ctx.enter_context(tc.tile_pool(name="vpool", bufs=2))
        vtp = ctx.enter_context(tc.tile_pool(name="vtp", bufs=1))
        ctxp = ctx.enter_context(tc.tile_pool(name="ctxp", bufs=2))
        expp = ctx.enter_context(
            tc.tile_pool(name="expp", bufs=48 if zero_bias else 40))
        dnp = ctx.enter_context(tc.tile_pool(name="dnp", bufs=2))
        outp = ctx.enter_context(tc.tile_pool(name="outp", bufs=4))
        psp = ctx.enter_context(tc.tile_pool(name="psp", bufs=1, space="PSUM"))

        def ps_tile(shape, tag, dtype=F32):
            return psp.tile(shape, dtype, tag=tag, name="ps_" + tag)

        # ---- constants: 1 big DMA per weight matrix (pre-swizzled on host) --
        wq_sb = consts.tile([128, NKT, DPC], BF16, tag="wq")
        wk_sb = consts.tile([128, NKT, DPC], BF16, tag="wk")
        wv_sb = consts.tile([128, NKT, DPC], BF16, tag="wv")
        nc.scalar.dma_start(out=wq_sb[:, :, :], in_=wq_d[:, :, :])
        nc.sync.dma_start(out=wk_sb[:, :, :], in_=wk_d[:, :, :])
        nc.sync.dma_start(out=wv_sb[:, :, :], in_=wv_d[:, :, :])
        wo_sb = consts.tile([128, D], BF16, tag="wo")
        nc.gpsimd.dma_start(out=wo_sb, in_=wo_d[:, :])
        bq_sb = consts.tile([128, 1], F32, tag="bq")
        bk_sb = consts.tile([128, 1], F32, tag="bk")
        bv_sb = consts.tile([128, 1], F32, tag="bv")
        nc.gpsimd.dma_start(out=bq_sb, in_=bq_d[:, :])
        nc.gpsimd.dma_start(out=bk_sb, in_=bk_d[:, :])
        nc.gpsimd.dma_start(out=bv_sb, in_=bv_d[:, :])
        ident_sb = consts.tile([128, 128], BF16, tag="ident")
        nc.gpsimd.dma_start(out=ident_sb, in_=ident_d[:, :])
        zero_sb = consts.tile([128, 1], F32, tag="zero")
        nc.vector.memset(zero_sb, 0.0)

        state = {}

        def load(b, engines):
            qt_sb = qt_pool.tile([128, NKT, S], BF16, tag="qt")
            i = 0
            for pc in range(2):
                for k in range(NKT):
                    engines[i % len(engines)].dma_start(
                        out=qt_sb[:, k, pc * 1024:(pc + 1) * 1024],
                        in_=qt_d[k * 128:(k + 1) * 128,
                                 b * S + pc * 1024: b * S + (pc + 1) * 1024])
                    i += 1
            state[b, "qt"] = qt_sb

        def alloc_proj(b):
            state[b, "K8"] = proj.tile([128, S], F8, tag="K8", name="K8")
            state[b, "Q8"] = proj.tile([128, 2, S], F8, tag="Q8", name="Q8")
            state[b, "VT"] = vtp.tile([128, S], BF16, tag="VT", name="VT")

        def proj_half(b, which, pc, hh, tag):
            """One [128, 512] projection half: 8 k-matmuls + evac."""
            qt_sb = state[b, "qt"]
            w_sb = {"q": wq_sb, "k": wk_sb, "v": wv_sb}[which]
            b_sb = {"q": bq_sb, "k": bk_sb, "v": bv_sb}[which]
            c0 = pc * 1024 + hh * 512
            ps = ps_tile([128, 512], tag)
            for k in range(NKT):
                nc.tensor.matmul(ps, w_sb[:, k, :], qt_sb[:, k, c0:c0 + 512],
                                 start=(k == 0), stop=(k == NKT - 1))
            sl = slice(c0, c0 + 512)
            if which == "k":
                if zero_bias:
                    nc.vector.tensor_copy(state[b, "K8"][:, sl], ps)
                else:
                    nc.vector.tensor_scalar(
                        out=state[b, "K8"][:, sl], in0=ps,
                        scalar1=b_sb, scalar2=None, op0=Alu.add)
            elif which == "v":
                if zero_bias:
                    nc.vector.tensor_copy(state[b, "VT"][:, sl], ps)
                else:
                    nc.vector.tensor_scalar(
                        out=state[b, "VT"][:, sl], in0=ps,
                        scalar1=b_sb, scalar2=None, op0=Alu.add)
            else:
                Q8 = state[b, "Q8"]
                if zero_bias:
                    nc.vector.tensor_copy(Q8[:, 0, sl], ps)
                    nc.vector.tensor_tensor(
                        out=Q8[:, 1, sl], in0=ps, in1=Q8[:, 0, sl],
                        op=Alu.subtract)
                else:
                    qtmp = outp.tile([128, 512], F32, tag="qtmp", name="qtmp")
                    nc.vector.tensor_scalar(
                        out=qtmp, in0=ps, scalar1=b_sb, scalar2=None, op0=Alu.add)
                    nc.vector.tensor_copy(Q8[:, 0, sl], qtmp)
                    nc.vector.tensor_tensor(
                        out=Q8[:, 1, sl], in0=qtmp, in1=Q8[:, 0, sl],
                        op=Alu.subtract)

        def alloc_v(b):
            V = vpool.tile([128, NST, 2, DH + 1], BF16, tag="V", name="V")
            nc.vector.memset(V[:, :, :, DH:DH + 1], 1.0)
            state[b, "V"] = V

        def tr_one(b, st, tag):
            VT, V = state[b, "VT"], state[b, "V"]
            ps_t = ps_tile([128, 128], tag, BF16)
            nc.tensor.transpose(ps_t, VT[:, st * 128:(st + 1) * 128], ident_sb)
            for u in range(2):
                nc.vector.tensor_copy(V[:, st, u, 0:DH], ps_t[:, u * DH:(u + 1) * DH])

        def outproj_st(b, st, tag, act_evac=False, fast_store=False, tags=None):
            ctxT = state[b, "ctxT"]
            o_sb = outp.tile([128, D], BF16, tag="o", name="o_sb")
            for oc in range(2):
                if tags is not None:
                    ps = ps_tile([128, 512], tags[oc])
                else:
                    ps = ps_tile([128, 512], tag if oc == 0 else
                                 ("pB" if tag == "pA" else "pA"))
                nc.tensor.matmul(ps,
                                 ctxT[:, st * 128:(st + 1) * 128],
                                 wo_sb[:, oc * 512:(oc + 1) * 512],
                                 start=True, stop=True)
                if act_evac and oc == 1:
                    nc.scalar.activation(o_sb[:, oc * 512:(oc + 1) * 512], ps,
                                         Act.Copy, bias=0.0, scale=1.0)
                else:
                    nc.vector.tensor_copy(o_sb[:, oc * 512:(oc + 1) * 512], ps)
            if fast_store:
                eng = nc.sync if st % 2 == 0 else nc.scalar
            else:
                eng = nc.sync if st % 2 == 0 else nc.gpsimd
            eng.dma_start(
                out=out_d[b * S + st * 128: b * S + (st + 1) * 128, :], in_=o_sb)

        def alloc_attn(b):
            state[b, "ctxT"] = ctxp.tile([128, S], BF16, tag="ctxT", name="ctxT")
            state[b, "denom"] = dnp.tile([1, 2, S], BF16, tag="denom", name="denom")

        def scores(b, qc, u, sk):
            K8, Q8 = state[b, "K8"], state[b, "Q8"]
            ps = ps_tile([128, 1024], "sA" if u == 0 else "sB")
            state[b, qc, u, "ps"] = ps
            lhsT = _dup2(K8[u * DH:(u + 1) * DH, sk * 128:(sk + 1) * 128])
            for hh in range(2):
                c0 = qc * 1024 + hh * 512
                nc.tensor.matmul(
                    ps[:, hh * 512:(hh + 1) * 512],
                    lhsT, Q8[u * DH:(u + 1) * DH, :, c0:c0 + 512],
                    start=True, stop=True, perf_mode=DR)

        def expop(b, qc, u, sk):
            e = expp.tile([128, 1024], BF16, tag="exp", name="exp_t")
            nc.scalar.activation(e, state[b, qc, u, "ps"], Act.Exp,
                                 bias=zero_sb, scale=1.0)
            state[b, qc, u, sk] = e

        def ctx2(pb, pqc, hh, sk, tags=("cA", "cB")):
            """One sk step of the hh-half ctx accumulation for qc (pb,pqc)."""
            V = state[pb, "V"]
            for u in range(2):
                key = (pb, pqc, u, "pc", hh)
                if sk == 0:
                    state[key] = ps_tile([DH + 1, 512], tags[u])
                e = state[pb, pqc, u, sk]
                nc.tensor.matmul(
                    state[key], V[:, sk, u, :],
                    e[:, hh * 512:(hh + 1) * 512],
                    start=(sk == 0), stop=(sk == NST - 1))

        def evac_half(pb, pqc, hh):
            ctxT, denom = state[pb, "ctxT"], state[pb, "denom"]
            sl = slice(pqc * 1024 + hh * 512, pqc * 1024 + hh * 512 + 512)
            for u in range(2):
                pc = state[pb, pqc, u, "pc", hh]
                nc.vector.tensor_copy(ctxT[u * DH:(u + 1) * DH, sl], pc[0:DH, :])
                nc.vector.tensor_copy(denom[0:1, u, sl], pc[DH:DH + 1, :])

        def normalize(pb, pqc, hh=None, eng=None):
            eng = eng or nc.gpsimd
            ctxT, denom = state[pb, "ctxT"], state[pb, "denom"]
            if hh is None:
                sl = slice(pqc * 1024, (pqc + 1) * 1024)
                w = 1024
            else:
                sl = slice(pqc * 1024 + hh * 512, pqc * 1024 + hh * 512 + 512)
                w = 512
            eng.dma_start(out=dn_d[:, sl], in_=denom[0:1, :, sl])
            key = "rep"
            if key not in state:
                state[key] = dnp.tile([128, 1024], BF16, tag="repb", name="repb")
                state["repf"] = dnp.tile([128, 1024], BF16, tag="rep", name="rep")
            rep, repf = state[key], state["repf"]
            for u in range(2):
                src_ap = dn_d[u:u + 1, sl]
                eng.dma_start(
                    out=repf[u * DH:(u + 1) * DH, 0:w],
                    in_=bass.AP(tensor=src_ap.tensor, offset=src_ap.offset,
                                ap=[[0, DH]] + [list(p) for p in src_ap.ap[1:]]))
            nc.vector.reciprocal(rep[:, 0:w], repf[:, 0:w])
            nc.vector.tensor_tensor(out=ctxT[:, sl], in0=ctxT[:, sl],
                                    in1=rep[:, 0:w], op=Alu.mult)

        def thunk(f, *a):
            def g():
                f(*a)
            return g

        def unit(b, qc, prev, inserts, extra_ctx=()):
            """Scores+exp of (b, qc); ctx of prev woven in 3 thunks/iter;
            one insert thunk per iter (None = skip)."""
            inserts = deque(inserts)
            ctx_work = deque()
            if prev is not None:
                pb, pqc = prev
                for hh in range(2):
                    for sk in range(NST):
                        ctx_work.append(thunk(ctx2, pb, pqc, hh, sk))
                    ctx_work.append(thunk(evac_half, pb, pqc, hh))
                ctx_work.append(thunk(normalize, pb, pqc))
            ctx_work.extend(extra_ctx)
            scores(b, qc, 0, 0)
            scores(b, qc, 1, 0)
            for sk in range(NST):
                expop(b, qc, 0, sk)
                expop(b, qc, 1, sk)
                if sk + 1 < NST:
                    scores(b, qc, 0, sk + 1)
                for _ in range(3):
                    if ctx_work:
                        ctx_work.popleft()()
                if sk + 1 < NST:
                    scores(b, qc, 1, sk + 1)
                if inserts:
                    ins = inserts.popleft()
                    if ins is not None:
                        ins()
            while ctx_work:
                ctx_work.popleft()()
            while inserts:
                ins = inserts.popleft()
                if ins is not None:
                    ins()

        def proj_pair(b, w1, w2, pc, hh):
            """Two interleaved [128,512] projections sharing qt k-tiles."""
            qt_sb = state[b, "qt"]
            wsb = {"q": wq_sb, "k": wk_sb, "v": wv_sb}
            c0 = pc * 1024 + hh * 512
            psA = ps_tile([128, 512], "pA")
            psB = ps_tile([128, 512], "pB")
            for k in range(NKT):
                nc.tensor.matmul(psA, wsb[w1][:, k, :], qt_sb[:, k, c0:c0 + 512],
                                 start=(k == 0), stop=(k == NKT - 1))
                nc.tensor.matmul(psB, wsb[w2][:, k, :], qt_sb[:, k, c0:c0 + 512],
                                 start=(k == 0), stop=(k == NKT - 1))
            for which, ps in ((w1, psA), (w2, psB)):
                sl = slice(c0, c0 + 512)
                if which == "k":
                    nc.vector.tensor_copy(state[b, "K8"][:, sl], ps)
                elif which == "v":
                    nc.vector.tensor_copy(state[b, "VT"][:, sl], ps)
                else:
                    Q8 = state[b, "Q8"]
                    nc.vector.tensor_copy(Q8[:, 0, sl], ps)
                    nc.vector.tensor_tensor(
                        out=Q8[:, 1, sl], in0=ps, in1=Q8[:, 0, sl],
                        op=Alu.subtract)

        # =========================== schedule ===========================
        load(0, (nc.sync, nc.scalar))
        alloc_proj(0)
        alloc_v(0)
        # PE p-state warm-up: dummy matmuls on a memset tile keep the engine
        # busy through the qt DMA wait so real projections run at higher clock
        warm = consts.tile([128, 512], BF16, tag="warm")
        nc.vector.memset(warm[:, 0:128], 0.0)
        wps = ps_tile([128, 512], "sA")
        for i in range(18):
            nc.tensor.matmul(wps, warm[:, 0:128], warm[:, :],
                             start=(i == 0), stop=(i == 17))
        if zero_bias:
            proj_pair(0, "k", "q", 0, 0)
            proj_pair(0, "q", "k", 0, 1)
        else:
            proj_half(0, "k", 0, 0, "pA")
            proj_half(0, "k", 0, 1, "pB")
            proj_half(0, "q", 0, 0, "pA")
            proj_half(0, "q", 0, 1, "pB")
        alloc_attn(0)

        def tr2(b, s0):
            tr_one(b, s0, "pA")
            tr_one(b, s0 + 1, "pB")

        def q1_and_load1(b, hh):
            proj_half(b, "q", 1, hh, "pB" if hh else "pA")
            if hh == 1:
                load(1, (nc.sync, nc.gpsimd))

        unit(0, 0, None, [
            thunk(proj_half, 0, "v", 0, 0, "pA"),
            thunk(proj_half, 0, "v", 0, 1, "pB"),
            thunk(tr2, 0, 0), thunk(tr2, 0, 2), thunk(tr2, 0, 4), thunk(tr2, 0, 6),
            thunk(proj_half, 0, "k", 1, 0, "pA"),
            thunk(proj_half, 0, "k", 1, 1, "pB"),
            thunk(proj_half, 0, "v", 1, 0, "pA"),
            thunk(proj_half, 0, "v", 1, 1, "pB"),
            thunk(tr2, 0, 8), thunk(tr2, 0, 10), thunk(tr2, 0, 12), thunk(tr2, 0, 14),
            thunk(q1_and_load1, 0, 0),
            thunk(q1_and_load1, 0, 1),
        ])

        def op4(b, s0):
            for i in range(4):
                outproj_st(b, s0 + i, "pA" if i % 2 == 0 else "pB")

        alloc_proj(1)
        alloc_v(1)
        unit(0, 1, (0, 0), [
            None, None, None, None, None, None, None, None,
            thunk(proj_half, 1, "k", 0, 0, "pA"),
            thunk(proj_half, 1, "k", 0, 1, "pB"),
            thunk(proj_half, 1, "q", 0, 0, "pA"),
            thunk(proj_half, 1, "q", 0, 1, "pB"),
            thunk(op4, 0, 0), thunk(op4, 0, 4),
            thunk(proj_half, 1, "v", 0, 0, "pA"),
            thunk(proj_half, 1, "v", 0, 1, "pB"),
        ])

        alloc_attn(1)
        unit(1, 0, (0, 1), [
            thunk(tr2, 1, 0), thunk(tr2, 1, 2), thunk(tr2, 1, 4), thunk(tr2, 1, 6),
            thunk(proj_half, 1, "k", 1, 0, "pA"),
            thunk(proj_half, 1, "k", 1, 1, "pB"),
            thunk(proj_half, 1, "v", 1, 0, "pA"),
            thunk(proj_half, 1, "v", 1, 1, "pB"),
            thunk(tr2, 1, 8), thunk(tr2, 1, 10), thunk(tr2, 1, 12), thunk(tr2, 1, 14),
            thunk(proj_half, 1, "q", 1, 0, "pA"),
            thunk(proj_half, 1, "q", 1, 1, "pB"),
            thunk(op4, 0, 8), thunk(op4, 0, 12),
        ])

        # last unit: ctx(1,1)-hh0 runs as the per-iter insert stream in pA/pB
        # (sk lags exp production by one iter); hh1 appended to ctx_work
        unit(1, 1, (1, 0),
             [None] + [thunk(ctx2, 1, 1, 0, sk, ("pA", "pB")) for sk in range(NST - 1)],
             extra_ctx=[thunk(ctx2, 1, 1, 1, sk) for sk in range(14)])

        # tail: close out, both normalize halves on parallel queues, outproj
        ctx2(1, 1, 0, NST - 1, ("pA", "pB"))
        ctx2(1, 1, 1, 14)
        ctx2(1, 1, 1, 15)
        evac_half(1, 1, 0)
        evac_half(1, 1, 1)
        normalize(1, 1, hh=0, eng=nc.sync)
        normalize(1, 1, hh=1, eng=nc.scalar)
        rot = [("pA", "pB"), ("sA", "sB"), ("cA", "cB"), ("pB", "pA"),
               ("sB", "sA"), ("cB", "cA")]
        for st in range(0, NST):
            outproj_st(1, st, "pA", act_evac=(st % 2 == 1), fast_store=True,
                       tags=rot[st % len(rot)])

    _split_sync_commands(nc)
    return nc


def _prepare(query, q_w, q_b, k_w, k_b, v_w, v_b, out_w):
    bf16 = ml_dtypes.bfloat16
    qt = np.ascontiguousarray(query.reshape(BS, D).T).astype(bf16)  # [D, BS]

    def swizzle(wt):
        # [D, DPC] -> SBUF layout [128, NKT, DPC]: arr[p, k, c] = wt[k*128+p, c]
        return np.ascontiguousarray(
            wt.reshape(NKT, 128, DPC).transpose(1, 0, 2)).astype(bf16)

    in_maps = []
    for c in range(N_CORES):
        sl = slice(c * DPC, (c + 1) * DPC)
        in_maps.append({
            "qt": qt,
            "wq": swizzle(q_w[sl, :].T * 0.125),
            "wk": swizzle(k_w[sl, :].T),
            "wv": swizzle(v_w[sl, :].T),
            "bq": np.ascontiguousarray((q_b[sl] * 0.125).reshape(DPC, 1)),
            "bk": np.ascontiguousarray(k_b[sl].reshape(DPC, 1)),
            "bv": np.ascontiguousarray(v_b[sl].reshape(DPC, 1)),
            "wo": np.ascontiguousarray(out_w[:, sl].T).astype(bf16),
        })
    return in_maps


def kernel(query, mask, q_w, q_b, k_w, k_b, v_w, v_b, out_w, out_b):
    query = np.asarray(query, dtype=np.float32)
    q_w = np.asarray(q_w, dtype=np.float32); q_b = np.asarray(q_b, dtype=np.float32)
    k_w = np.asarray(k_w, dtype=np.float32); k_b = np.asarray(k_b, dtype=np.float32)
    v_w = np.asarray(v_w, dtype=np.float32); v_b = np.asarray(v_b, dtype=np.float32)
    out_w = np.asarray(out_w, dtype=np.float32); out_b = np.asarray(out_b, dtype=np.float32)

    zero_bias = not (np.any(q_b) or np.any(k_b) or np.any(v_b))
    in_maps = _prepare(query, q_w, q_b, k_w, k_b, v_w, v_b, out_w)
    nc = _build(zero_bias)
    res = run_bass_kernel_spmd(nc, in_maps, core_ids=list(range(N_CORES)))
    out = np.zeros((BS, D), dtype=np.float32)
    for c in range(N_CORES):
        out += res.results[c]["out_part"].astype(np.float32)
    out += out_b[None, :]
    return out.reshape(B, S, D)
